# revision 84
# baseline (speedup 1.0000x reference)
import sys

sys.path.insert(0, "/opt/trn_rl_repo")
from contextlib import ExitStack

import numpy as np

from concourse import bass, bacc, tile
from concourse.bass_utils import run_bass_kernel_spmd
from concourse.masks import make_identity

mybir = bass.mybir
AF = mybir.ActivationFunctionType
ALU = mybir.AluOpType
F32 = mybir.dt.float32
F32R = mybir.dt.float32r
BF16 = mybir.dt.bfloat16
F16 = mybir.dt.float16
I16 = mybir.dt.int16
U16 = mybir.dt.uint16

B = 8
N = 8192
S = 2048
DF = 256
DL = 128
O0 = 256
O1 = 128
NCHUNK = N // 128
NSLICE = N // 512
EPS_W = 1e-8
EPS_BN = 1e-5
INV_TOT = 1.0 / (B * N)

_CACHE = {}


def _split3(nc, src_f32, parts_bf, tmp1, tmp2, prange=None):
    # parts_bf: list of 3 bf16 AP destinations, same shape as src
    nc.scalar.activation(parts_bf[0], src_f32, AF.Copy)
    nc.vector.tensor_tensor(tmp1, src_f32, parts_bf[0], ALU.subtract)
    nc.scalar.activation(parts_bf[1], tmp1, AF.Copy)
    nc.vector.tensor_tensor(tmp2, tmp1, parts_bf[1], ALU.subtract)
    nc.scalar.activation(parts_bf[2], tmp2, AF.Copy)


def _build():
    nc = bacc.Bacc("TRN2", target_bir_lowering=False, debug=False, num_devices=B)

    xyzl_h = nc.dram_tensor("xyzl", [N, 3], F32, kind="ExternalInput")
    xyzf_h = nc.dram_tensor("xyzf", [S, 3], F32, kind="ExternalInput")
    featf_h = nc.dram_tensor("featf", [S, DF], F32, kind="ExternalInput")
    featl_h = nc.dram_tensor("featl", [N, DL], F32, kind="ExternalInput")
    w0_h = nc.dram_tensor("w0", [O0, 384], F32, kind="ExternalInput")
    w1_h = nc.dram_tensor("w1", [O1, O0], F32, kind="ExternalInput")
    g0_h = nc.dram_tensor("g0", [O0], F32, kind="ExternalInput")
    bt0_h = nc.dram_tensor("bt0", [O0], F32, kind="ExternalInput")
    g1_h = nc.dram_tensor("g1", [O1], F32, kind="ExternalInput")
    bt1_h = nc.dram_tensor("bt1", [O1], F32, kind="ExternalInput")
    out_h = nc.dram_tensor("out", [O1, N], F32, kind="ExternalOutput")

    featf_ap = featf_h.ap()

    with tile.TileContext(nc) as tc:
        with ExitStack() as ctx:
            sb = ctx.enter_context(tc.tile_pool(name="sb", bufs=1))
            sb2 = ctx.enter_context(tc.tile_pool(name="sb2", bufs=3))
            sbs = ctx.enter_context(tc.tile_pool(name="sbs", bufs=3))
            sbg = ctx.enter_context(tc.tile_pool(name="sbg", bufs=2))
            sb3 = ctx.enter_context(tc.tile_pool(name="sb3", bufs=3))
            sbtr = ctx.enter_context(tc.tile_pool(name="sbtr", bufs=2))
            sbx = ctx.enter_context(tc.tile_pool(name="sbx", bufs=2))
            ps_d = ctx.enter_context(tc.tile_pool(name="psd", bufs=3, space="PSUM"))
            ps_t = ctx.enter_context(tc.tile_pool(name="pst", bufs=1, space="PSUM"))
            ps_y = ctx.enter_context(tc.tile_pool(name="psy", bufs=1, space="PSUM"))
            dr = ctx.enter_context(tc.tile_pool(name="dr", bufs=1, space="DRAM"))

            ident = sb.tile([128, 128], F32)
            make_identity(nc, ident[:, :])
            epsb = sb.tile([128, 1], F32)
            nc.vector.memset(epsb[:, :], EPS_BN)
            z256 = sb.tile([128, 256], F32)
            nc.vector.memset(z256[:, :], 0.0)
            id16b = sb.tile([16, 16], F16)
            nc.scalar.activation(id16b[:, :], ident[0:16, 0:16], AF.Copy)
            rep16 = sb.tile([16, 128], F16)
            for rj in range(8):
                nc.sync.dma_start(rep16[:, 16 * rj:16 * (rj + 1)], id16b[:, :])

            y1 = sb.tile([128, N], F32)

            # ---- load points into blocked layouts ----
            # p_sb[c*32+blk, j] = xyzl[blk*256+j, c]
            p_sb = sb.tile([96, 256], F32)
            pxyz = y1[0:96, 0:768]
            for c in range(3):
                nc.sync.dma_start(
                    y1[c * 32:(c + 1) * 32, 0:766],
                    bass.AP(xyzl_h, c, [[768, 32], [1, 766]]),
                )
            nc.scalar.activation(
                p_sb[:, :],
                bass.AP(pxyz.tensor, pxyz.offset, [[8192, 96], [3, 256]]),
                AF.Copy,
            )
            q_sb = sb.tile([24, 256], F32)
            for c in range(3):
                nc.sync.dma_start(
                    y1[c * 8:(c + 1) * 8, 1024:1790],
                    bass.AP(xyzf_h, c, [[768, 8], [1, 766]]),
                )
            nc.scalar.activation(
                q_sb[:, :],
                bass.AP(y1.tensor, y1.offset + 1024, [[8192, 24], [3, 256]]),
                AF.Copy,
            )

            # ---- split3 of 2p and q ----
            p2 = sb.tile([96, 256], F32)
            nc.scalar.activation(p2[:, :], p_sb[:, :], AF.Copy, scale=2.0)
            pa = sb.tile([96, 256], BF16)
            pb = sb.tile([96, 256], BF16)
            pc = sb.tile([96, 256], BF16)
            pr1 = sb.tile([96, 256], F32)
            pr2 = sb.tile([96, 256], F32)
            _split3(nc, p2[:, :], [pa[:, :], pb[:, :], pc[:, :]], pr1[:, :], pr2[:, :])
            qa = sb.tile([24, 256], BF16)
            qb = sb.tile([24, 256], BF16)
            qc = sb.tile([24, 256], BF16)
            qr1 = sb.tile([24, 256], F32)
            qr2 = sb.tile([24, 256], F32)
            _split3(nc, q_sb[:, :], [qa[:, :], qb[:, :], qc[:, :]], qr1[:, :], qr2[:, :])

            # ---- norms + their split3 (base-0 tiles to satisfy BIR partition rule) ----
            sq_p = [sb.tile([32, 256], F32, name=f"sqp{c}") for c in range(3)]
            for c in range(3):
                nc.scalar.activation(sq_p[c][:, :], p_sb[c * 32:(c + 1) * 32, :], AF.Square)
            pn2 = sb.tile([32, 256], F32)
            nc.vector.tensor_tensor(pn2[:, :], sq_p[0][:, :], sq_p[1][:, :], ALU.add)
            nc.vector.tensor_tensor(pn2[:, :], pn2[:, :], sq_p[2][:, :], ALU.add)
            pns_t = [sb.tile([32, 256], BF16, name=f"pns{s}") for s in range(3)]
            prn1 = sb.tile([32, 256], F32)
            prn2 = sb.tile([32, 256], F32)
            _split3(nc, pn2[:, :], [t[:, :] for t in pns_t], prn1[:, :], prn2[:, :])

            sq_q_all = sb.tile([24, 256], F32)
            nc.scalar.activation(sq_q_all[:, :], q_sb[:, :], AF.Square)
            sq_q1 = sb.tile([8, 256], F32)
            nc.sync.dma_start(sq_q1[:, :], sq_q_all[8:16, :])
            sq_q2 = sb.tile([8, 256], F32)
            nc.sync.dma_start(sq_q2[:, :], sq_q_all[16:24, :])
            qn2 = sb.tile([8, 256], F32)
            nc.vector.tensor_tensor(qn2[:, :], sq_q_all[0:8, :], sq_q1[:, :], ALU.add)
            nc.vector.tensor_tensor(qn2[:, :], qn2[:, :], sq_q2[:, :], ALU.add)
            qns_t = [sb.tile([8, 256], BF16, name=f"qns{s}") for s in range(3)]
            qrn1 = sb.tile([8, 256], F32)
            qrn2 = sb.tile([8, 256], F32)
            _split3(nc, qn2[:, :], [t[:, :] for t in qns_t], qrn1[:, :], qrn2[:, :])

            # ---- pack lhsT_all [24, 32, 256] and rhs_all [24, 8, 256] ----
            lhsT_all = sb.tile([24, 32, 256], BF16)
            rhs_all = sb.tile([24, 8, 256], BF16)
            neg1 = sb.tile([3, 8, 256], BF16)
            nc.vector.memset(neg1[:, :, :], -1.0)
            for q4 in range(4):
                nc.sync.dma_start(lhsT_all[18:21, q4 * 8:(q4 + 1) * 8, :], neg1[:, :, :])
            nc.sync.dma_start(rhs_all[21:24, :, :], neg1[:, :, :])
            for g, srct in [(0, pa), (3, pa), (6, pb), (9, pa), (12, pc), (15, pb)]:
                nc.sync.dma_start(lhsT_all[g:g + 3, :, :], srct[:, :])
            for s in range(3):
                nc.sync.dma_start(lhsT_all[21 + s:22 + s, :, :], pns_t[s][:, :])
            for g, srct in [(0, qa), (3, qb), (6, qa), (9, qc), (12, qa), (15, qb)]:
                nc.sync.dma_start(rhs_all[g:g + 3, :, :], srct[:, :])
            for s in range(3):
                nc.sync.dma_start(rhs_all[18 + s:19 + s, :, :], qns_t[s][:, :])

            # ---- weights: transpose W0, W1 ----
            w0_sb = [y1[:, 2048 + ob * 384:2048 + (ob + 1) * 384] for ob in range(2)]
            for ob in range(2):
                nc.sync.dma_start(
                    w0_sb[ob][:, :],
                    bass.AP(w0_h, ob * 128 * 384, [[384, 128], [1, 384]]),
                )
            w0t = [sb.tile([128, 256], BF16, name=f"w0t{cb}") for cb in range(3)]
            for cb in range(3):
                for ob in range(2):
                    ptw = ps_t.tile([128, 384], F32, tag="pt", name="ptw")
                    nc.tensor.transpose(ptw[:, 0:128], w0_sb[ob][:, cb * 128:(cb + 1) * 128], ident[:, :])
                    nc.scalar.activation(w0t[cb][:, ob * 128:(ob + 1) * 128], ptw[:, 0:128], AF.Copy)
            w1_sb = y1[:, 2816:3072]
            nc.sync.dma_start(w1_sb[:, :], w1_h.ap())
            w1t = [sb.tile([128, 128], BF16, name=f"w1t{cb}") for cb in range(2)]
            for cb in range(2):
                ptw = ps_t.tile([128, 384], F32, tag="pt", name="ptw")
                nc.tensor.transpose(ptw[:, 0:128], w1_sb[:, cb * 128:(cb + 1) * 128], ident[:, :])
                nc.scalar.activation(w1t[cb][:, :], ptw[:, 0:128], AF.Copy)

            featl_sb = sb.tile([128, NCHUNK * DL], F32)
            for fp in range(8):
                nc.sync.dma_start(
                    featl_sb[:, fp * 8 * DL:(fp + 1) * 8 * DL],
                    bass.AP(featl_h, fp * 8 * 128 * DL,
                            [[DL, 128], [DL * 128, 8], [1, DL]]),
                )

            g0b = sb.tile([128, 2], F32)
            nc.sync.dma_start(g0b[:, :], bass.AP(g0_h, 0, [[1, 128], [128, 2]]))
            b0b = sb.tile([128, 2], F32)
            nc.sync.dma_start(b0b[:, :], bass.AP(bt0_h, 0, [[1, 128], [128, 2]]))
            g1b = sb.tile([128, 1], F32)
            nc.sync.dma_start(g1b[:, :], g1_h.ap())
            b1b = sb.tile([128, 1], F32)
            nc.sync.dma_start(b1b[:, :], bt1_h.ap())

            # ---- persistent activations + stats ----
            y0a = sb.tile([128, N], BF16)
            y0b = sb.tile([128, N], BF16)
            s0a = sb.tile([128, NSLICE], F32)
            s0b = sb.tile([128, NSLICE], F32)
            q0a = sb.tile([128, NSLICE], F32)
            q0b = sb.tile([128, NSLICE], F32)
            s1 = sb.tile([128, NSLICE], F32)
            q1 = sb.tile([128, NSLICE], F32)

            # ---- main chunk loop ----
            def emit_y0(jn, xTt):
                for ob in range(2):
                    py = ps_y.tile([128, 512], F32, tag="py", name="py")
                    for cb in range(3):
                        nc.tensor.matmul(
                            py[:, :],
                            w0t[cb][:, ob * 128:(ob + 1) * 128],
                            xTt[:, cb, :],
                            start=(cb == 0), stop=(cb == 2),
                        )
                    y0t = y0a if ob == 0 else y0b
                    s0t = s0a if ob == 0 else s0b
                    q0t = q0a if ob == 0 else q0b
                    nc.scalar.activation(
                        y0t[:, jn * 512:(jn + 1) * 512], py[:, :], AF.Copy,
                        accum_out=s0t[:, jn:jn + 1],
                    )
                    trash = sbtr.tile([128, 512], BF16, tag="trash", name="trash")
                    nc.scalar.activation(
                        trash[:, :], py[:, :], AF.Square,
                        accum_out=q0t[:, jn:jn + 1],
                    )

            pending_y0 = None
            for i in range(NCHUNK):
                jn, t = divmod(i, 4)
                blk, half = divmod(i, 2)
                lhs_chunk = lhsT_all[:, blk, half * 128:half * 128 + 128]

                dsb = sb3.tile([128, S], F32, tag="dsb", name="dsb")
                for h in range(2):
                    pd = ps_d.tile([128, 1024], F32, tag="pd", name="pd")
                    for m in range(2):
                        nc.tensor.matmul(
                            pd[:, m * 512:(m + 1) * 512], lhs_chunk,
                            rhs_all[:, h * 4 + m * 2:h * 4 + m * 2 + 2, :],
                            start=True, stop=True,
                        )
                    nc.scalar.activation(dsb[:, h * 1024:(h + 1) * 1024], pd[:, :], AF.Copy)

                if pending_y0 is not None:
                    emit_y0(*pending_y0)
                    pending_y0 = None

                pair, side = divmod(i, 2)
                if side == 0:
                    idx = sbs.tile([128, 128], U16, tag="idx", name="idx")
                    idxw = sbs.tile([128, 48], I16, tag="idxw", name="idxw")
                    gf = sbg.tile([128, 6, DF], F32, tag="gf", name="gf")
                maxv = sbs.tile([128, 8], F32, tag="maxv", name="maxv")
                nc.vector.max(maxv[:, :], dsb[:, :])
                nc.vector.max_index(idx[:, side * 8:side * 8 + 8], maxv[:, :], dsb[:, :])

                # weights: w3 = normalize(1 / (-maxv3 + eps))
                d3 = sbs.tile([128, 3], F32, tag="d3", name="d3")
                nc.scalar.activation(d3[:, :], maxv[:, 0:3], AF.Copy, bias=EPS_W, scale=-1.0)
                rec = sbs.tile([128, 3], F32, tag="rec", name="rec")
                nc.vector.reciprocal(rec[:, :], d3[:, :])
                junk3 = sbs.tile([128, 3], F32, tag="junk3", name="junk3")
                rsum = sbs.tile([128, 1], F32, tag="rsum", name="rsum")
                nc.scalar.activation(junk3[:, :], rec[:, :], AF.Copy, accum_out=rsum[:, :])
                rsi = sbs.tile([128, 1], F32, tag="rsi", name="rsi")
                nc.vector.reciprocal(rsi[:, :], rsum[:, :])
                w3 = sbs.tile([128, 3], F32, tag="w3", name="w3")
                nc.scalar.activation(w3[:, :], rec[:, :], AF.Copy, scale=rsi[:, 0:1])

                if side == 0:
                    w3_a = w3

                if side == 1:
                    # wrapped gather indices for the PAIR: convert idx to f16,
                    # XBAR transpose + packs + DVE 32x32 transposes, then
                    # replicate across the 8 16-partition stripes with a
                    # one-hot PE matmul instead of 16 small DMAs.
                    idxf = sbs.tile([128, 128], F16, tag="idxf", name="idxf")
                    nc.scalar.activation(idxf[:, 0:16], idx[:, 0:16], AF.Copy)
                    i3t = sbs.tile([128, 128], I16, tag="i3t", name="i3t")
                    nc.sync.dma_start(
                        i3t[:, :], idxf[:, :].bitcast(I16), transpose=True
                    )
                    pbc = ps_t.tile([128, 512], F32, tag="pt", name="pbc")
                    for sd in range(2):
                        vpk = sbs.tile([32, 32], I16, tag=f"vpk{sd}", name="vpk")
                        nc.sync.dma_start(
                            vpk[0:24, 0:16],
                            bass.AP(i3t.tensor, sd * 8 * 128, [[128, 3], [16, 8], [1, 16]]),
                        )
                        w32 = sbs.tile([32, 32], I16, tag=f"w32{sd}", name="w32")
                        nc.vector.transpose(w32[:, :], vpk[:, :])
                        nc.tensor.matmul(
                            pbc[:, sd * 24:sd * 24 + 24], rep16[:, :],
                            w32[0:16, 0:24].bitcast(F16),
                            start=True, stop=True,
                        )
                    nc.scalar.activation(idxw[:, :], pbc[:, 0:48], AF.Copy)
                    nc.gpsimd.dma_gather(gf[:, :, :], featf_ap, idxw[:, :], 768, 768, DF)

                    # deferred interp + transposes for both chunks of the pair
                    for ci, w3c in ((i - 1, w3_a), (i, w3)):
                        sd = ci & 1
                        tc_ = ci % 4
                        jn_ = ci // 4
                        itp = sb2.tile([128, DF], F32, tag="itp", name="itp")
                        tacc = sb2.tile([128, DF], F32, tag="tacc", name="tacc")
                        nc.vector.scalar_tensor_tensor(
                            tacc[:, :], gf[:, sd * 3 + 1, :], w3c[:, 1:2], z256[:, :],
                            ALU.mult, ALU.add,
                        )
                        nc.vector.scalar_tensor_tensor(
                            tacc[:, :], gf[:, sd * 3 + 2, :], w3c[:, 2:3], tacc[:, :],
                            ALU.mult, ALU.add,
                        )
                        nc.scalar.activation(
                            itp[:, :], gf[:, sd * 3, :], AF.Copy, scale=w3c[:, 0:1]
                        )
                        nc.gpsimd.tensor_tensor(itp[:, :], itp[:, :], tacc[:, :], ALU.add)

                        fl = featl_sb[:, ci * DL:(ci + 1) * DL]
                        if tc_ == 0:
                            xTt = sbx.tile([128, 3, 512], BF16, tag="xt", name="xt")
                        ptx = ps_t.tile([128, 512], F32, tag="pt", name="ptx")
                        nc.tensor.transpose(ptx[:, 0:128], fl, ident[:, :])
                        nc.tensor.transpose(ptx[:, 128:256], itp[:, 0:128], ident[:, :])
                        nc.tensor.transpose(ptx[:, 256:384], itp[:, 128:256], ident[:, :])
                        nc.scalar.activation(
                            xTt[:, :, tc_ * 128:(tc_ + 1) * 128], ptx[:, 0:384], AF.Copy
                        )
                        if tc_ == 3:
                            pending_y0 = (jn_, xTt)

            if pending_y0 is not None:
                emit_y0(*pending_y0)
                pending_y0 = None

            # ---- BN0 stats (cross-device) ----
            st0 = sb.tile([128, 4], F32)
            nc.vector.tensor_reduce(st0[:, 0:1], s0a[:, :], mybir.AxisListType.X, ALU.add)
            nc.vector.tensor_reduce(st0[:, 1:2], s0b[:, :], mybir.AxisListType.X, ALU.add)
            nc.vector.tensor_reduce(st0[:, 2:3], q0a[:, :], mybir.AxisListType.X, ALU.add)
            nc.vector.tensor_reduce(st0[:, 3:4], q0b[:, :], mybir.AxisListType.X, ALU.add)
            cin0 = dr.tile([128, 4], F32)
            cout0 = dr.tile([8, 128, 4], F32, addr_space="Shared")
            nc.sync.dma_start(cin0[:, :], st0[:, :])
            nc.gpsimd.collective_compute(
                "AllGather", ALU.bypass,
                replica_groups=[list(range(B))],
                ins=[cin0.opt()], outs=[cout0.opt()],
            )
            stg0 = sb.tile([128, 32], F32)
            nc.sync.dma_start(
                stg0[:, :], bass.AP(cout0.tensor, cout0.offset, [[4, 128], [512, 8], [1, 4]])
            )
            str0 = sb.tile([128, 4], F32)
            nc.vector.tensor_reduce(
                str0[:, :],
                bass.AP(stg0.tensor, stg0.offset, [[32, 128], [1, 4], [4, 8]]),
                mybir.AxisListType.X, ALU.add,
            )

            mn0 = sb.tile([128, 4], F32)
            nc.scalar.activation(mn0[:, :], str0[:, :], AF.Copy, scale=INV_TOT)
            var0 = sb.tile([128, 2], F32)
            nc.vector.tensor_tensor(var0[:, :], mn0[:, 0:2], mn0[:, 0:2], ALU.mult)
            nc.vector.tensor_tensor(var0[:, :], mn0[:, 2:4], var0[:, :], ALU.subtract)
            std0 = sb.tile([128, 2], F32)
            nc.scalar.activation(std0[:, :], var0[:, :], AF.Sqrt, bias=epsb[:, 0:1])
            rstd0 = sb.tile([128, 2], F32)
            nc.vector.reciprocal(rstd0[:, :], std0[:, :])
            a0 = sb.tile([128, 2], F32)
            nc.vector.tensor_tensor(a0[:, :], g0b[:, :], rstd0[:, :], ALU.mult)
            c0 = sb.tile([128, 2], F32)
            nc.vector.tensor_tensor(c0[:, :], mn0[:, 0:2], a0[:, :], ALU.mult)
            nc.vector.tensor_tensor(c0[:, :], b0b[:, :], c0[:, :], ALU.subtract)
            for ob, y0t in ((0, y0a), (1, y0b)):
                for sl in range(8):
                    seg = y0t[:, sl * 1024:(sl + 1) * 1024]
                    if sl >= 2:
                        nc.vector.tensor_scalar(
                            seg, seg, a0[:, ob:ob + 1], c0[:, ob:ob + 1],
                            ALU.mult, ALU.add,
                        )
                        nc.vector.tensor_scalar_max(seg, seg, 0.0)
                    else:
                        nc.scalar.activation(
                            seg, seg,
                            AF.Relu, bias=c0[:, ob:ob + 1], scale=a0[:, ob:ob + 1],
                        )

            # ---- layer 1 ----
            for jn in range(NSLICE):
                py = ps_t.tile([128, 512], F32, tag="pt", name="py1")
                nc.tensor.matmul(
                    py[:, :], w1t[0][:, :],
                    y0a[:, jn * 512:(jn + 1) * 512],
                    start=True, stop=False,
                )
                nc.tensor.matmul(
                    py[:, :], w1t[1][:, :],
                    y0b[:, jn * 512:(jn + 1) * 512],
                    start=False, stop=True,
                )
                nc.scalar.activation(
                    y1[:, jn * 512:(jn + 1) * 512], py[:, :], AF.Copy,
                    accum_out=s1[:, jn:jn + 1],
                )
                trash = sbtr.tile([128, 512], BF16, tag="trash", name="trash1")
                ysl = y1[:, jn * 512:(jn + 1) * 512]
                nc.vector.scalar_tensor_tensor(
                    trash[:, :], ysl, 1.0, ysl,
                    ALU.mult, ALU.mult, accum_out=q1[:, jn:jn + 1],
                )

            # ---- BN1 ----
            st1 = sb.tile([128, 2], F32)
            nc.vector.tensor_reduce(st1[:, 0:1], s1[:, :], mybir.AxisListType.X, ALU.add)
            nc.vector.tensor_reduce(st1[:, 1:2], q1[:, :], mybir.AxisListType.X, ALU.add)
            cin1 = dr.tile([128, 2], F32)
            cout1 = dr.tile([8, 128, 2], F32, addr_space="Shared")
            nc.sync.dma_start(cin1[:, :], st1[:, :])
            nc.gpsimd.collective_compute(
                "AllGather", ALU.bypass,
                replica_groups=[list(range(B))],
                ins=[cin1.opt()], outs=[cout1.opt()],
            )
            stg1 = sb.tile([128, 16], F32)
            nc.sync.dma_start(
                stg1[:, :], bass.AP(cout1.tensor, cout1.offset, [[2, 128], [256, 8], [1, 2]])
            )
            str1 = sb.tile([128, 2], F32)
            nc.vector.tensor_reduce(
                str1[:, :],
                bass.AP(stg1.tensor, stg1.offset, [[16, 128], [1, 2], [2, 8]]),
                mybir.AxisListType.X, ALU.add,
            )

            mn1 = sb.tile([128, 2], F32)
            nc.scalar.activation(mn1[:, :], str1[:, :], AF.Copy, scale=INV_TOT)
            var1 = sb.tile([128, 1], F32)
            nc.vector.tensor_tensor(var1[:, :], mn1[:, 0:1], mn1[:, 0:1], ALU.mult)
            nc.vector.tensor_tensor(var1[:, :], mn1[:, 1:2], var1[:, :], ALU.subtract)
            std1 = sb.tile([128, 1], F32)
            nc.scalar.activation(std1[:, :], var1[:, :], AF.Sqrt, bias=epsb[:, 0:1])
            rstd1 = sb.tile([128, 1], F32)
            nc.vector.reciprocal(rstd1[:, :], std1[:, :])
            a1 = sb.tile([128, 1], F32)
            nc.vector.tensor_tensor(a1[:, :], g1b[:, :], rstd1[:, :], ALU.mult)
            c1 = sb.tile([128, 1], F32)
            nc.vector.tensor_tensor(c1[:, :], mn1[:, 0:1], a1[:, :], ALU.mult)
            nc.vector.tensor_tensor(c1[:, :], b1b[:, :], c1[:, :], ALU.subtract)
            for sl in range(8):
                seg = y1[:, sl * 1024:(sl + 1) * 1024]
                if sl % 2 == 1:
                    nc.vector.tensor_scalar(
                        seg, seg, a1[:, 0:1], c1[:, 0:1], ALU.mult, ALU.add,
                    )
                    nc.vector.tensor_scalar_max(seg, seg, 0.0)
                else:
                    nc.scalar.activation(
                        seg, seg, AF.Relu, bias=c1[:, 0:1], scale=a1[:, 0:1],
                    )
                nc.sync.dma_start(
                    bass.AP(out_h, sl * 1024, [[N, 128], [1, 1024]]),
                    seg,
                )
    nc.compile()
    return nc


def kernel(**inputs):
    xyzl = np.ascontiguousarray(np.asarray(inputs["point_xyz_large"], dtype=np.float32))
    xyzf = np.ascontiguousarray(np.asarray(inputs["point_xyz_few"], dtype=np.float32))
    featf = np.ascontiguousarray(np.asarray(inputs["point_feature_few"], dtype=np.float32))
    featl = np.ascontiguousarray(np.asarray(inputs["point_feature_large"], dtype=np.float32))
    w0 = np.ascontiguousarray(np.asarray(inputs["W0"], dtype=np.float32))
    w1 = np.ascontiguousarray(np.asarray(inputs["W1"], dtype=np.float32))
    g0 = np.ascontiguousarray(np.asarray(inputs["g0"], dtype=np.float32))
    bt0 = np.ascontiguousarray(np.asarray(inputs["beta0"], dtype=np.float32))
    g1 = np.ascontiguousarray(np.asarray(inputs["g1"], dtype=np.float32))
    bt1 = np.ascontiguousarray(np.asarray(inputs["beta1"], dtype=np.float32))

    if "nc" not in _CACHE:
        _CACHE["nc"] = _build()
    nc = _CACHE["nc"]

    in_maps = [
        {
            "xyzl": xyzl[b], "xyzf": xyzf[b], "featf": featf[b], "featl": featl[b],
            "w0": w0, "w1": w1, "g0": g0, "bt0": bt0, "g1": g1, "bt1": bt1,
        }
        for b in range(B)
    ]
    res = run_bass_kernel_spmd(
        nc, in_maps, list(range(B)), trace=_CACHE.get("trace", False)
    )
    out = np.stack([np.asarray(res.results[b]["out"]) for b in range(B)], 0)
    _CACHE["last_res"] = res
    return out.astype(np.float32)

